# revision 24
# baseline (speedup 1.0000x reference)
"""LDW-upsample (lifting wavelet) kernel for 8 Trainium2 NeuronCores.

The reference module is linear in x:
    out[b, j, 2h+r, 2w+s] = sum_c Weff_{r,s}[j, c] * x[b, c, h, w]
where Weff folds the 1x1-conv weight and the 4 lifting filter taps, so the
whole module is one 256->256 1x1 conv + a 2x2 pixel-shuffle.

Sharding: pure data parallel, 2 batch images per core.

The fp32 version of this kernel sits exactly on the DMA roofline
(67 MB/core / 332 GB/s = 202 us), so this version moves all device I/O to
bf16 (x cast on host, output upcast on host): 33.6 MB/core -> ~101 us
floor, and the matmuls run at bf16 rate (1 cycle/row vs 2-4 for fp32).

Per-core dataflow (raw bass, 4 engines, manual semaphores):
  - PSUM output partitions carry m = r*64 + j (row-parity packed with the
    channel) so matmuls are full M=128; one PSUM bank per (s, b), double
    buffered over super-chunks (8 banks exactly).
  - SP (HWDGE queue): input DMAs (2 MiB bf16 blocks of 16 input rows, both
    images+k-tiles merged in one 3-dim AP) into a 6-slot ring. Block 0
    lands in 4 chunks so PE starts ~4 us early; loads are paced ~4 blocks
    ahead of the output stream so the two DMA flows interleave instead of
    inputs hogging the DMA engines. SP also flushes the LAST block's
    output in row-halves as soon as each half is evicted (shorter drain).
  - PE: bf16 matmuls, k accumulated in PSUM; 8 matmuls per super-chunk.
  - ACT evicts b=0's PSUM banks (both s), DVE evicts b=1's (fp32 -> bf16
    cast in the copy), stride-2 free-dim writes interleaving s into a
    shared 4-slot SBUF ring. The split is by b, NOT by s: the s=0/s=1 bf16
    elements of one image share 32-bit SBUF words, and two engines doing
    sub-word RMW writes to the same words would race.
  - ACT (own HWDGE queue): 4 output DMAs per block (one per (b, r), 64
    partitions, 512 B contiguous bursts).

DMA-completion semaphores are EXCLUSIVE per ring slot: a DMA's
.then_inc(sem, 16) arrives as 16 piecemeal per-engine increments, so a
cumulative threshold shared by several in-flight transfers can fire on a
mix of partial completions (this was a real, observed ~50%-flaky data
race). Every wait below is on a semaphore whose only contributors are the
transfers the waiter needs in full.

TimelineSim (local cost model): 97.4 us/rep; DMA-engine occupancy 96%,
which is the roofline for 2x16.78 MB at 360 GB/s. (The fp32 baseline
measured 205 us on HW, exactly its own DMA roofline, so the model's DMA
side tracks hardware well.) Measured on HW: rel err 3.757e-3,
deterministic across 6 consecutive runs.
"""

import sys

for _p in ("/opt/trn_rl_repo",):
    if _p not in sys.path:
        sys.path.insert(0, _p)

import numpy as np
import ml_dtypes

B, C, H, W = 16, 256, 128, 128
C4 = C // 4
N_CORES = 8
B_PER_CORE = B // N_CORES  # 2
H2, W2 = 2 * H, 2 * W

POS_PER_IMG = H * W  # 16384
BLK_POS = 2048  # input pixels per block (16 input rows), per image
BLK_ROWS = BLK_POS // W  # 16
N_BLK = POS_PER_IMG // BLK_POS  # 8 (each block covers BOTH images)
SC_POS = 512  # super-chunk pixels (4 input rows): one PSUM bank per (s,b)
SC_PER_BLK = BLK_POS // SC_POS  # 4
N_SC = N_BLK * SC_PER_BLK  # 32

IN_SLOT = 4 * BLK_POS  # 8192 elements per in_buf slot (b,k merged)
IN_SLOTS = 6  # deep input ring (paced by output progress)
OUT_SLOT = 2 * 4096  # per out_buf slot: [b(2), rr(16), x(256)]
OUT_SLOTS = 4

_CACHE = {}


def _effective_weights(conv1x1_w, lp_v, hp_v, lp_h, hp_h):
    """Fold lifting taps into the conv weight.

    Returns w_all bf16 [128, 512]: four lhsT tiles side by side, tile
    index s*2+k, each [c_in_ktile(128 part), m(128)] with m = r*64 + j,
    computed in f64.
    """
    Wd = conv1x1_w.astype(np.float64)
    lv = lp_v.reshape(C4, 2).astype(np.float64)
    hv = hp_v.reshape(C4, 2).astype(np.float64)
    lh = lp_h.reshape(C4, 2).astype(np.float64)
    hh = hp_h.reshape(C4, 2).astype(np.float64)

    va = np.stack([lv[:, 0], hv[:, 0]], axis=1)  # [j, r]
    vb = np.stack([lv[:, 1], hv[:, 1]], axis=1)
    hc0 = np.stack([lh[:, 0], hh[:, 0]], axis=1)  # [j, s]
    hc1 = np.stack([lh[:, 1], hh[:, 1]], axis=1)

    Wa, Wb, Wc, Wdq = Wd[:C4], Wd[C4 : 2 * C4], Wd[2 * C4 : 3 * C4], Wd[3 * C4 :]

    weff = {}
    for r in (0, 1):
        for s in (0, 1):
            weff[(r, s)] = (
                (hc0[:, s] * va[:, r])[:, None] * Wa
                + (hc0[:, s] * vb[:, r])[:, None] * Wb
                + (hc1[:, s] * va[:, r])[:, None] * Wc
                + (hc1[:, s] * vb[:, r])[:, None] * Wdq
            )  # [j, c]

    tiles = []
    for s in (0, 1):
        for k in (0, 1):
            cols = [weff[(r, s)][:, k * 128 : (k + 1) * 128].T for r in (0, 1)]
            tiles.append(np.concatenate(cols, axis=1))  # [c(128), m(128)]
    w_all = np.concatenate(tiles, axis=1)  # [128, 512]
    return np.ascontiguousarray(w_all).astype(ml_dtypes.bfloat16)


def _build_nc(reps=1):
    """reps>1 repeats the whole pipeline (same data) inside one NEFF --
    benchmarking only, to scale the HW-exec signal above dispatch noise."""
    import concourse.bass as bass
    import concourse.mybir as mybir

    f32 = mybir.dt.float32
    bf16 = mybir.dt.bfloat16
    nc = bass.Bass()

    xs = nc.declare_dram_parameter("xs", [B_PER_CORE, C, H, W], bf16, isOutput=False)
    wp = nc.declare_dram_parameter("w", [128, 512], bf16, isOutput=False)
    ys = nc.declare_dram_parameter("ys", [B_PER_CORE, C4, H2, W2], bf16, isOutput=True)

    # Input view per block q: partition p = channel-within-ktile; free dims
    # (bk = b*2+k merged by uniform stride, pos contiguous).
    xv = xs[:].rearrange("b (k p) (q hh) w -> q p (b k) (hh w)", k=2, hh=BLK_ROWS)
    # Output view per (block q, image b, parity r): partition j, free
    # (hh: stride 2 rows, x contiguous 256 = one full output row).
    yv = ys[:].rearrange("b j (q hh r) x -> q b r j hh x", hh=BLK_ROWS, r=2)
    # same but with the 16 hh rows split in two halves of 8
    yvh = ys[:].rearrange(
        "b j (q h2 hh r) x -> q h2 b r j hh x", h2=2, hh=BLK_ROWS // 2, r=2
    )

    from contextlib import ExitStack

    with ExitStack() as ctx:
        ec = ctx.enter_context
        w_all = ec(nc.sbuf_tensor("w_all", [128, 512], bf16))
        in_buf = ec(nc.sbuf_tensor("in_buf", [128, IN_SLOTS * IN_SLOT], bf16))
        out_buf = ec(nc.sbuf_tensor("out_buf", [128, OUT_SLOTS * OUT_SLOT], bf16))
        # ps[s][b][slot]
        ps = [
            [
                [ec(nc.psum_tensor(f"ps{s}{b}{sl}", [128, SC_POS], f32)) for sl in (0, 1)]
                for b in (0, 1)
            ]
            for s in (0, 1)
        ]
        w_sem = ec(nc.semaphore("w_sem"))
        # Exclusive DMA-completion semaphores: a DMA's 16 increments arrive
        # piecemeal (one per DMA engine share), so a cumulative threshold can
        # be crossed by a MIX of partial completions from several in-flight
        # transfers. Each semaphore below is only ever incremented by
        # transfers the waiter needs in full, so >= 16*n is exact.
        isem = [ec(nc.semaphore(f"is{m}")) for m in range(IN_SLOTS)]
        csem = [ec(nc.semaphore(f"c{m}")) for m in (1, 2, 3)]
        osem = [ec(nc.semaphore(f"os{m}")) for m in range(OUT_SLOTS)]
        mmA_sem = ec(nc.semaphore("mmA_sem"))
        mmV_sem = ec(nc.semaphore("mmV_sem"))
        evA_sem = ec(nc.semaphore("evA_sem"))
        evV_sem = ec(nc.semaphore("evV_sem"))
        block = ec(nc.Block())

        def wtile(s, k):
            i = s * 2 + k
            return w_all[:, i * 128 : (i + 1) * 128]

        def rhs(t, b, k, off):
            base = (t % IN_SLOTS) * IN_SLOT + (b * 2 + k) * BLK_POS + off
            return in_buf[:, base : base + SC_POS]

        # out_buf as [p, slot(2), b(2), rr(16), w(128), s(2)];
        # partition p = r*64 + j; output row within block = 2*rr + r.
        obv = out_buf[:].rearrange(
            "p (slot b rr w s) -> p slot b rr w s", slot=OUT_SLOTS, b=2, rr=BLK_ROWS, s=2
        )

        NB = N_BLK * reps
        NSC = N_SC * reps

        def outdma(eng, t, oslot, b):
            base = oslot * OUT_SLOT + b * 4096
            for r in (0, 1):
                sb = out_buf[r * 64 : (r + 1) * 64, base : base + 4096].rearrange(
                    "p (hh x) -> p hh x", x=W2
                )
                eng.dma_start(out=yv[t % N_BLK, b, r], in_=sb).then_inc(
                    osem[oslot], 16
                )

        @block.sync
        def _(sync: "bass.BassEngine"):
            # SP owns the input stream (HWDGE queue, in-order completion)
            # plus the one-off weight load. Block 0 is split in SC-sized
            # chunks so the PE can start ~4us earlier.
            sync.dma_start(out=w_all[:], in_=wp[:]).then_inc(w_sem, 16)
            for t in range(NB):
                if t >= 4:
                    # Pace input ~4 blocks ahead of the output stream so the
                    # two DMA flows interleave instead of inputs hogging the
                    # DMA engines up front and jamming the out_buf recycle.
                    sync.wait_ge(osem[t % OUT_SLOTS], 64 * (t // OUT_SLOTS))
                if t >= IN_SLOTS:
                    # in_buf slot reuse: PE finished reading block t-IN_SLOTS
                    sync.wait_ge(mmV_sem, SC_PER_BLK * (t - IN_SLOTS + 1))
                base = (t % IN_SLOTS) * IN_SLOT
                if t == 0:
                    iv = in_buf[:, base : base + IN_SLOT].rearrange(
                        "p (bk pos) -> p bk pos", bk=4
                    )
                    for cc in range(SC_PER_BLK):
                        sync.dma_start(
                            out=iv[:, :, cc * SC_POS : (cc + 1) * SC_POS],
                            in_=xv[0][:, :, cc * SC_POS : (cc + 1) * SC_POS],
                        ).then_inc(isem[0] if cc == 0 else csem[cc - 1], 16)
                else:
                    sync.dma_start(
                        out=in_buf[:, base : base + IN_SLOT],
                        in_=xv[t % N_BLK],
                    ).then_inc(isem[t % IN_SLOTS], 16)
            # Last block: flush each half as soon as its evictions land so
            # the drain tail is ~2 half-DMAs instead of 4 full ones. 8 half
            # transfers inc osem by 128 total.
            tl = NB - 1
            olast = tl % OUT_SLOTS
            base0 = olast * OUT_SLOT
            for half in (0, 1):
                sync.wait_ge(evA_sem, SC_PER_BLK * tl + 2 * (half + 1))
                sync.wait_ge(evV_sem, SC_PER_BLK * tl + 2 * (half + 1))
                for b in (0, 1):
                    for r in (0, 1):
                        hb = base0 + b * 4096 + half * 2048
                        sb = out_buf[
                            r * 64 : (r + 1) * 64, hb : hb + 2048
                        ].rearrange("p (hh x) -> p hh x", x=W2)
                        sync.dma_start(
                            out=yvh[tl % N_BLK, half, b, r], in_=sb
                        ).then_inc(osem[olast], 16)
            for m in range(OUT_SLOTS):
                n_full = len([t for t in range(NB - 1) if t % OUT_SLOTS == m])
                sync.wait_ge(osem[m], 64 * n_full + (128 if m == olast else 0))

        @block.tensor
        def _(tensor: "bass.BassEngine"):
            tensor.wait_ge(w_sem, 16)
            for sc in range(NSC):
                t, cc = divmod(sc, SC_PER_BLK)
                slot = sc % 2
                if sc < SC_PER_BLK:
                    # block 0 arrives in sc-sized chunks, one sem per chunk
                    tensor.wait_ge(isem[0] if cc == 0 else csem[cc - 1], 16)
                elif cc == 0:
                    tensor.wait_ge(isem[t % IN_SLOTS], 16 * (t // IN_SLOTS + 1))
                if sc >= 2:
                    # PSUM slot reuse: evictions of super-chunk sc-2 done
                    tensor.wait_ge(evA_sem, sc - 1)
                    tensor.wait_ge(evV_sem, sc - 1)
                off = cc * SC_POS
                for s, sem in ((0, mmA_sem), (1, mmV_sem)):
                    last = None
                    for k in (0, 1):
                        for b in (0, 1):
                            last = tensor.matmul(
                                ps[s][b][slot][:, :],
                                lhsT=wtile(s, k),
                                rhs=rhs(t, b, k, off),
                                start=(k == 0),
                                stop=(k == 1),
                            )
                    last.then_inc(sem, 1)

        @block.scalar
        def _(scalar: "bass.BassEngine"):
            # ACT owns image b=0: evicts its s=0 and s=1 banks (sequential
            # sub-word writes from ONE engine are safe), and issues the
            # output DMAs on its own HWDGE queue, independent of SP's input
            # queue.
            for sc in range(NSC):
                t, cc = divmod(sc, SC_PER_BLK)
                slot = sc % 2  # PSUM double-buffer slot
                oslot = t % OUT_SLOTS  # out_buf ring slot
                if cc == 0 and t >= OUT_SLOTS:
                    # out_buf slot reuse: block t-OUT_SLOTS's out DMAs done
                    scalar.wait_ge(osem[t % OUT_SLOTS], 64 * (t // OUT_SLOTS))
                scalar.wait_ge(mmA_sem, sc + 1)
                src = ps[0][0][slot][:].rearrange("p (h w) -> p h w", w=W)
                scalar.copy(out=obv[:, oslot, 0, cc * 4 : (cc + 1) * 4, :, 0], in_=src)
                scalar.wait_ge(mmV_sem, sc + 1)
                src = ps[1][0][slot][:].rearrange("p (h w) -> p h w", w=W)
                ev = scalar.copy(
                    out=obv[:, oslot, 0, cc * 4 : (cc + 1) * 4, :, 1], in_=src
                )
                ev.then_inc(evA_sem, 1)
                if cc == SC_PER_BLK - 1 and t != NB - 1:
                    # b=0 rows of this block are complete: issue its DMAs,
                    # then b=1's once DVE's evictions are done. (The last
                    # block is flushed in halves from SP instead.)
                    outdma(scalar, t, oslot, 0)
                    scalar.wait_ge(evV_sem, SC_PER_BLK * (t + 1))
                    outdma(scalar, t, oslot, 1)

        @block.vector
        def _(vector: "bass.BassEngine"):
            # DVE owns image b=1.
            for sc in range(NSC):
                t, cc = divmod(sc, SC_PER_BLK)
                slot = sc % 2  # PSUM double-buffer slot
                oslot = t % OUT_SLOTS  # out_buf ring slot
                if cc == 0 and t >= OUT_SLOTS:
                    vector.wait_ge(osem[t % OUT_SLOTS], 64 * (t // OUT_SLOTS))
                vector.wait_ge(mmA_sem, sc + 1)
                src = ps[0][1][slot][:].rearrange("p (h w) -> p h w", w=W)
                vector.tensor_copy(obv[:, oslot, 1, cc * 4 : (cc + 1) * 4, :, 0], src)
                vector.wait_ge(mmV_sem, sc + 1)
                src = ps[1][1][slot][:].rearrange("p (h w) -> p h w", w=W)
                ev = vector.tensor_copy(
                    obv[:, oslot, 1, cc * 4 : (cc + 1) * 4, :, 1], src
                )
                ev.then_inc(evV_sem, 1)

    return nc


def _get_nc(reps=1):
    key = ("nc", reps)
    if key not in _CACHE:
        _CACHE[key] = _build_nc(reps)
    return _CACHE[key]


def run_on_cores(x, w_all, trace=False):
    from concourse.bass_utils import run_bass_kernel_spmd

    nc = _get_nc()
    xb = np.ascontiguousarray(x, dtype=np.float32).astype(ml_dtypes.bfloat16)
    in_maps = [
        {
            "xs": xb[i * B_PER_CORE : (i + 1) * B_PER_CORE],
            "w": w_all,
        }
        for i in range(N_CORES)
    ]
    res = run_bass_kernel_spmd(nc, in_maps, list(range(N_CORES)), trace=trace)
    out = np.concatenate([res.results[i]["ys"] for i in range(N_CORES)], axis=0)
    return out.astype(np.float32), res


def kernel(x, conv1x1_w, lp_v, hp_v, lp_h, hp_h):
    w_all = _effective_weights(
        np.asarray(conv1x1_w),
        np.asarray(lp_v),
        np.asarray(hp_v),
        np.asarray(lp_h),
        np.asarray(hp_h),
    )
    out, _ = run_on_cores(np.asarray(x), w_all, trace=False)
    return out
